# revision 21
# baseline (speedup 1.0000x reference)
"""Trainium2 Bass kernel for nn_LC_Block (gnn_message_passing).

Strategy (pure data-parallel over batch, 2 batches/core on 8 cores):
  - BN1 folded into conv1 weights; temporal conv as Toeplitz matmul on PE
    in bf16 (fp32 PSUM accumulate). Conv bias pre-shifted by +1 so PSUM
    holds x+1.
  - ELU as elu(x)+1 = max(min(exp(x),1), x+1): ACT computes exp(x) from
    x+1 via bias=-1, one fused DVE scalar_tensor_tensor produces elu+1 in
    bf16. The +1 is linear through stage 2 and folds into the BN2 bias.
  - GCN + residual + depthwise-expansion conv + BN2 folded host-side into
    16 accumulating PE matmul weights (single el1 stream, f-major layout).
  - 3 PSUM buffers for the conv->exp->elu pipeline so the 3-engine chain
    is throughput- not latency-bound; PE prewarmed with dummy matmuls
    during the input DMA to release the HAM clock throttle early.
  - BN2 bias folded into the ACT Identity drain (accum_out gives the t-sum
    for channel attention for free). Tail kept in bf16 for 2x DVE modes.
  - Spatial attention c2-max via PE: per-125-col chunk transpose matmul
    (rhs=I), DVE grouped max-reduce, PE gather matmul back to row layout.
    No GPSIMD on the critical path. 3-tap conv as two accumulation groups
    (mean rows / max rows) on PE.
  - Only t<896 can affect the pooled output: h5/ELU/pool/sep-conv tail all
    trimmed accordingly; sep-conv as a fused DVE MAC chain on 48 cols.
"""
import numpy as np
import concourse.bass as bass
import concourse.tile as tile
import concourse.mybir as mybir
import concourse.bass_isa as bass_isa
from concourse.bass_utils import run_bass_kernel_spmd

F32 = mybir.dt.float32
BF16 = mybir.dt.bfloat16
AOP = mybir.AluOpType
AFT = mybir.ActivationFunctionType

B, F1, D, C, T, K = 16, 16, 2, 64, 1000, 64
C2 = F1 * D          # 32
PW = K // 4          # 16
EPS = 1e-5
NCORE = 8
BPC = B // NCORE     # 2
TBLK = 64
NBLK = 16            # covers t 0..1023 (1000 valid)
NF = F1 * TBLK       # 1024 conv out cols per block
TE = 896             # last t that can affect the pooled output


# ----------------------------------------------------------------- host prep
def _host_consts(inp):
    f = {}
    g1, b1, m1, v1 = (np.asarray(inp[k], np.float32) for k in ('g1', 'b1', 'm1', 'v1'))
    inv1 = g1 / np.sqrt(v1 + EPS)
    w1 = np.asarray(inp['conv1_w'], np.float32)[:, 0, 0, :]
    w1p = w1 * inv1[:, None]
    # +1 shift: PSUM holds x+1 so the fused ELU+1 needs no extra op
    b1p = (np.asarray(inp['conv1_b'], np.float32) - m1) * inv1 + b1 + 1.0

    Wt = np.zeros((128, NF), np.float32)
    for toff in range(TBLK):
        for ff in range(F1):
            Wt[toff:toff + K, ff * TBLK + toff] = w1p[ff]
    Wt[127, :] = np.repeat(b1p, TBLK)
    f['wtoep'] = Wt

    a_hat = np.asarray(inp['a_hat'], np.float32)
    dw_w = np.asarray(inp['dw_w'], np.float32)
    gcn_w = np.asarray(inp['gcn_w'], np.float32)
    gcn_b = np.asarray(inp['gcn_b'], np.float32)
    g2, b2, m2, v2 = (np.asarray(inp[k], np.float32) for k in ('g2', 'b2', 'm2', 'v2'))
    s2 = g2 / np.sqrt(v2 + EPS)
    G = (gcn_w[:, None, None] / F1) * np.einsum('fdc,cj->fdj', dw_w, a_hat)
    Kmat = np.zeros((F1, C, C2), np.float32)          # [f', j, c2]
    for ff in range(F1):
        for d in range(D):
            Kmat[ff, :, ff * D + d] += dw_w[ff, d, :]
    for ff in range(F1):
        for d in range(D):
            Kmat[:, :, ff * D + d] += G[ff, d, :][None, :]
    Kmat *= s2[None, None, :]
    Kstk = np.zeros((128, F1, 2 * C2), np.float32)    # [(b,j), f', (b,c2)]
    for b in range(BPC):
        Kstk[b * C:(b + 1) * C, :, b * C2:(b + 1) * C2] = np.transpose(Kmat, (1, 0, 2))
    f['kstk'] = Kstk.reshape(128, F1 * 2 * C2)        # [128, 1024]

    sw = dw_w.sum(-1)
    dw_b = np.asarray(inp['dw_b'], np.float32)
    # -Kmat.sum((0,1)) corrects for the +1 in el1 = elu+1
    bias2 = (s2 * (dw_b + np.repeat(gcn_b, D) * sw.reshape(-1) - m2) + b2
             - Kmat.sum((0, 1)))

    ca_w1 = np.asarray(inp['ca_w1'], np.float32)
    ca_b1 = np.asarray(inp['ca_b1'], np.float32)
    ca_w2 = np.asarray(inp['ca_w2'], np.float32)
    ca_b2 = np.asarray(inp['ca_b2'], np.float32)
    H = ca_w1.shape[0]
    lca1 = np.zeros((2 * C2, BPC * H), np.float32)
    lca2 = np.zeros((BPC * H, 2 * C2), np.float32)
    for b in range(BPC):
        lca1[b * C2:(b + 1) * C2, b * H:(b + 1) * H] = ca_w1.T
        lca2[b * H:(b + 1) * H, b * C2:(b + 1) * C2] = ca_w2.T

    sa_w = np.asarray(inp['sa_w'], np.float32)
    w6 = sa_w[0, :, 1, :]
    w6adj = w6.copy()
    w6adj[0] /= C2
    # diag-pair 3-tap weights: mean rows and max rows as separate groups
    lsam = np.zeros((2, 6), np.float32)
    lsax = np.zeros((2, 6), np.float32)
    for dt in range(3):
        for b in range(BPC):
            lsam[b, 2 * dt + b] = w6adj[0, dt]
            lsax[b, 2 * dt + b] = w6adj[1, dt]
    sa_g, sa_b, sa_m, sa_v = (float(np.asarray(inp[k]).reshape(-1)[0])
                              for k in ('sa_g', 'sa_b', 'sa_m', 'sa_v'))
    ssa = sa_g / np.sqrt(sa_v + EPS)
    bsa = sa_b - sa_m * ssa

    sep_w = np.asarray(inp['sep_w'], np.float32)[:, 0, 0, :]
    sep_b = np.asarray(inp['sep_b'], np.float32)
    g3, b3, m3, v3 = (np.asarray(inp[k], np.float32) for k in ('g3', 'b3', 'm3', 'v3'))
    s3 = g3 / np.sqrt(v3 + EPS)
    wsep = np.tile(sep_w * s3[:, None] / PW, (BPC, 1))          # [64, 16]
    bsep = np.tile(s3 * (sep_b - m3) + b3, BPC)

    # fp32 const pack [64, 90]:
    # 0 bias2 | 1:5 lca1 | 5 bca1[0:4] | 6 bca2 | 7 ssa[0:2] | 8 bsa[0:2]
    # | 9:25 wsep | 25 bsep | 26:90 lca2[0:4]
    p32 = np.zeros((64, 90), np.float32)
    p32[:, 0] = np.tile(bias2, BPC)
    p32[:, 1:5] = lca1
    p32[0:4, 5] = np.tile(ca_b1, BPC)
    p32[:, 6] = np.tile(ca_b2, BPC)
    p32[0:2, 7] = ssa
    p32[0:2, 8] = bsa
    p32[:, 9:25] = wsep
    p32[:, 25] = bsep
    p32[0:4, 26:90] = lca2
    f['p32'] = p32

    # bf16 const pack [64, 78]:
    # 0:2 lmean | 2:8 lsam[0:2] | 8:72 lbc[0:2] | 72:78 lsax[0:2]
    pb = np.zeros((64, 78), np.float32)
    for b in range(BPC):
        pb[b * C2:(b + 1) * C2, b] = 1.0              # lmean
    pb[0:2, 2:8] = lsam
    for b in range(BPC):
        pb[b, 8 + b * C2:8 + (b + 1) * C2] = 1.0      # lbc
    pb[0:2, 72:78] = lsax
    f['pb16'] = pb

    f['ident'] = np.eye(128, dtype=np.float32)
    dsep = np.zeros((64, 16 * 64), np.float32)
    for k in range(PW):
        dsep[np.arange(64), 64 * k + np.arange(64)] = wsep[:, k]
    f['dsep'] = dsep
    return f


def _host_xtiles(x, core):
    xc = np.asarray(x, np.float32)[core * BPC:(core + 1) * BPC, 0]  # [2, C, T]
    xTpad = np.zeros((NBLK * TBLK + 128, BPC * C), np.float32)
    xTpad[31:31 + T, :] = xc.reshape(BPC * C, T).T
    tiles = np.zeros((128, NBLK, BPC * C), np.float32)
    for i in range(NBLK):
        tiles[:, i, :] = xTpad[TBLK * i: TBLK * i + 128]
        tiles[127, i, :] = 1.0
    return tiles.reshape(128, NBLK * BPC * C)                       # [128, 2048]


# ------------------------------------------------------------- device program
_CACHE = {}


def _build_program():
    from concourse import bacc
    nc = bacc.Bacc("TRN2", target_bir_lowering=False, debug=False)
    di = lambda n, s, dt=F32: nc.dram_tensor(n, s, dt, kind="ExternalInput")
    wx_d = di("wx", [128, 3072], BF16)
    ks_d = di("kstk", [128, 1024], BF16)
    p32_d = di("p32", [64, 90])
    pbig_d = di("pbig", [128, 1230], BF16)
    out_d = nc.dram_tensor("out", [BPC, C2, 3], F32, kind="ExternalOutput")

    with tile.TileContext(nc) as tc:
        with (
            tc.tile_pool(name="sb", bufs=1) as sb,
            tc.tile_pool(name="ep", bufs=3) as ep,
            tc.tile_pool(name="cpsum", bufs=3, space="PSUM") as cpsum,
            tc.tile_pool(name="dpsum", bufs=2, space="PSUM") as dpsum,
        ):
            # ---- input loads. gpsimd SWDGE starts transfers earliest, so the
            # conv-critical xt chunk 0 + wtoep go there first.
            wt = sb.tile([128, NF], BF16, tag="wt", name="wt")
            x0a = sb.tile([128, 256], BF16, tag="x0a", name="x0a")
            x0b = sb.tile([128, 256], BF16, tag="x0b", name="x0b")
            xsb = [sb.tile([128, 512], BF16, tag=f"xt{q}", name=f"xt{q}")
                   for q in range(1, 4)]
            ks = sb.tile([128, 1024], BF16, tag="ks", name="ks")
            p32 = sb.tile([64, 90], F32, tag="p32", name="p32")
            pbig = sb.tile([128, 1230], BF16, tag="pbig", name="pbig")
            zt = sb.tile([128, 512], BF16, tag="zt", name="zt")
            nc.gpsimd.dma_start(wt[:, 0:512], wx_d.ap()[:, 0:512])
            nc.gpsimd.dma_start(x0a[:], wx_d.ap()[:, 1024:1280])
            nc.gpsimd.memset(zt[:], 0.0)
            nc.gpsimd.dma_start(wt[:, 512:1024], wx_d.ap()[:, 512:1024])
            nc.gpsimd.dma_start(x0b[:], wx_d.ap()[:, 1280:1536])
            for q in range(1, 4):
                nc.gpsimd.dma_start(xsb[q - 1][:],
                                    wx_d.ap()[:, 1024 + 512 * q:1536 + 512 * q])
            nc.scalar.dma_start(ks[:], ks_d.ap())
            nc.scalar.dma_start(p32[:], p32_d.ap())
            nc.scalar.dma_start(pbig[:], pbig_d.ap())
            ident = pbig[:, 0:128]
            dsep = pbig[0:64, 128:1152]
            pb = pbig[0:64, 1152:1230]

            b2t = p32[:, 0:1]
            lca1 = p32[:, 1:5]
            bca1 = p32[0:4, 5:6]
            bca2 = p32[:, 6:7]
            ssat = p32[0:2, 7:8]
            bsat = p32[0:2, 8:9]
            wsept = p32[:, 9:25]
            bsept = p32[:, 25:26]
            lca2 = p32[0:4, 26:90]
            lmt = pb[:, 0:2]
            lsam = pb[0:2, 2:8]
            lbct = pb[0:2, 8:72]
            lsax = pb[0:2, 72:78]

            # ---- PE prewarm: dummy matmuls on zeros while inputs stream in
            warm = dpsum.tile([128, 512], F32, tag="dp", name="warm")
            for _ in range(3):
                nc.tensor.matmul(warm[:], zt[:, 0:128], zt[:])

            # el1 split tiles: f-major layout (f, blk, toff) so stage-2 rhs
            # is contiguous per f'. Last two tiles hold 2 blocks each so the
            # final accumulation group (post stage-1) is short.
            ELB = (4, 4, 4, 2, 2)          # blocks per el1 tile
            ELO = (0, 4, 8, 12, 14)        # first block of each tile
            el1 = [sb.tile([128, nb * NF], BF16, tag=f"el1{q}",
                           name=f"el1{q}") for q, nb in enumerate(ELB)]
            neg1 = sb.tile([128, 1], F32, tag="neg1", name="neg1")
            nc.vector.memset(neg1[:], -1.0)
            nc.vector.memset(el1[4][:], 0.0)   # block-15 toff>=40 never written

            # ---- conv1 (+BN1, +1) + fused ELU+1, 16 blocks
            for i in range(NBLK):
                cp = cpsum.tile([128, NF], F32, tag="cp", name="cp")
                if i < 2:
                    lhs = x0a[:, 128 * i:128 * (i + 1)]
                elif i < 4:
                    lhs = x0b[:, 128 * (i - 2):128 * (i - 1)]
                else:
                    lhs = xsb[i // 4 - 1][:, 128 * (i % 4):128 * (i % 4 + 1)]
                nc.tensor.matmul(cp[:, 0:512], lhs, wt[:, 0:512])
                nc.tensor.matmul(cp[:, 512:1024], lhs, wt[:, 512:1024])
                e_t = ep.tile([128, NF], BF16, tag="e", name="e")
                qi = 3 if i in (12, 13) else (4 if i >= 14 else i // 4)
                ov = el1[qi][:].rearrange("p (f blk toff) -> p f blk toff",
                                          f=F1, blk=ELB[qi])[:, :, i - ELO[qi], :]
                if i == NBLK - 1:
                    cpv = cp[:].rearrange("p (f toff) -> p f toff",
                                          f=F1)[:, :, 0:40]
                    ev = e_t[:].rearrange("p (f toff) -> p f toff",
                                          f=F1)[:, :, 0:40]
                    nc.scalar.activation(ev, cpv, AFT.Exp, bias=neg1[:])
                    nc.vector.scalar_tensor_tensor(ov[:, :, 0:40], ev, 1.0,
                                                   cpv, op0=AOP.min,
                                                   op1=AOP.max)
                else:
                    nc.scalar.activation(e_t[:], cp[:], AFT.Exp, bias=neg1[:])
                    nc.vector.scalar_tensor_tensor(ov, e_t[:], 1.0, cp[:],
                                                   op0=AOP.min, op1=AOP.max)

            # ---- fused GCN + depthwise-expansion + BN2 matmuls (single stream)
            h3b = sb.tile([64, T], BF16, tag="h3b", name="h3b")
            casum = [sb.tile([64, 1], F32, tag=f"cas{h}", name=f"cas{h}")
                     for h in range(2)]
            HGRP = (((0, 0, 256), (1, 256, 512)),
                    ((2, 0, 256), (3, 256, 384), (4, 384, 512)))
            for h in range(2):
                dp = dpsum.tile([64, 512], F32, tag="dp", name="dp")
                for (q, c0, c1) in HGRP[h]:
                    wq = c1 - c0
                    for fp in range(F1):
                        nc.tensor.matmul(
                            dp[:, c0:c1],
                            ks[:, 64 * fp:64 * (fp + 1)],
                            el1[q][:, wq * fp:wq * (fp + 1)],
                            start=(fp == 0), stop=(fp == F1 - 1))
                w = 512 if h == 0 else T - 512
                nc.scalar.activation(h3b[:, 512 * h:512 * h + w], dp[:, 0:w],
                                     AFT.Identity, bias=b2t,
                                     accum_out=casum[h][:])

            # ---- channel attention (mean from casum, max via DVE reduce)
            hmax = sb.tile([64, 1], F32, tag="hmax", name="hmax")
            nc.vector.tensor_reduce(hmax[:], h3b[:], axis=mybir.AxisListType.X,
                                    op=AOP.max)
            s1 = sb.tile([64, 1], F32, tag="s1", name="s1")
            nc.vector.tensor_tensor(s1[:], casum[0][:], casum[1][:], op=AOP.add)
            s3t = sb.tile([64, 1], F32, tag="s3t", name="s3t")
            nc.vector.scalar_tensor_tensor(s3t[:], s1[:], 1.0 / T, hmax[:],
                                           op0=AOP.mult, op1=AOP.add)
            # sigmoid table preload, anchored after the h=1 drain
            dums = sb.tile([2, 1], F32, tag="dums", name="dums")
            nc.scalar.activation(dums[:], el1[4][0:2, 64:65], AFT.Sigmoid)
            p1 = dpsum.tile([4, 1], F32, tag="dp", name="p1")
            nc.tensor.matmul(p1[:], lca1, s3t[:])
            u = sb.tile([4, 1], F32, tag="u", name="u")
            nc.vector.tensor_scalar(u[:], p1[:], bca1, 0.0,
                                    op0=AOP.add, op1=AOP.max)
            p2 = dpsum.tile([64, 1], F32, tag="dp", name="p2")
            nc.tensor.matmul(p2[:], lca2, u[:])
            att = sb.tile([64, 1], F32, tag="att", name="att")
            nc.scalar.activation(att[:], p2[:], AFT.Sigmoid, bias=bca2)
            h4 = sb.tile([64, T], BF16, tag="h4", name="h4")
            nc.vector.tensor_scalar(h4[:], h3b[:], att[:], None, op0=AOP.mult)

            # ---- spatial attention
            # mean rows (b0,b1) in scpad; max rows (b0,b1) in scmax
            scpad = sb.tile([2, T + 2], BF16, tag="scpad", name="scpad")
            nc.vector.memset(scpad[:], 0.0)
            scmax = sb.tile([2, T + 2], BF16, tag="scmax", name="scmax")
            nc.vector.memset(scmax[:], 0.0)
            for (a, b) in ((0, 500), (500, 1000)):
                sp = dpsum.tile([2, 500], F32, tag="dp", name="sp")
                nc.tensor.matmul(sp[:, 0:b - a], lmt, h4[:, a:b])
                nc.scalar.activation(scpad[0:2, 1 + a:1 + b], sp[:, 0:b - a],
                                     AFT.Identity)
            # c2-max via PE transpose + DVE grouped reduce + PE gather
            msb = sb.tile([128, 16], BF16, tag="msb", name="msb")
            for c in range(8):
                t0 = 125 * c
                trp = cpsum.tile([125, 64], F32, tag="cp", name="trp")
                nc.tensor.matmul(trp[:], h4[:, t0:t0 + 125], ident[0:64, 0:64])
                nc.vector.tensor_reduce(
                    msb[0:125, 2 * c:2 * c + 2],
                    trp[:].rearrange("p (b c2) -> p b c2", b=2),
                    axis=mybir.AxisListType.X, op=AOP.max)
                asm = dpsum.tile([2, 125], F32, tag="dp", name="asm")
                nc.tensor.matmul(asm[:], msb[0:125, 2 * c:2 * c + 2],
                                 ident[0:125, 0:125])
                nc.scalar.activation(scmax[0:2, 1 + t0:1 + t0 + 125], asm[:],
                                     AFT.Identity)
            # 3-tap conv: two accumulation groups (mean rows, max rows)
            msa = sb.tile([2, TE], BF16, tag="msa", name="msa")
            for (a, b) in ((0, 496), (496, TE)):
                pp = dpsum.tile([2, 496], F32, tag="dp", name="pp")
                for dt in range(3):
                    nc.tensor.matmul(pp[:, 0:b - a], lsam[:, 2 * dt:2 * dt + 2],
                                     scpad[0:2, a + dt:b + dt],
                                     start=(dt == 0), stop=False)
                for dt in range(3):
                    nc.tensor.matmul(pp[:, 0:b - a], lsax[:, 2 * dt:2 * dt + 2],
                                     scmax[0:2, a + dt:b + dt],
                                     start=False, stop=(dt == 2))
                nc.scalar.activation(msa[:, a:b], pp[:, 0:b - a], AFT.Sigmoid,
                                     bias=bsat, scale=ssat)
            # exp table preload, anchored after the last sigmoid
            dume = sb.tile([2, 1], F32, tag="dume", name="dume")
            nc.scalar.activation(dume[:], msa[0:2, TE - 1:TE], AFT.Exp)

            # ---- h5 = h4 * sigmoid(sa); ELU via max(h5, min(exp,1)-1); pool16
            h5 = sb.tile([64, TE], BF16, tag="h5", name="h5")
            e5 = sb.tile([64, TE], BF16, tag="e5", name="e5")
            q5 = sb.tile([64, TE], BF16, tag="q5", name="q5")
            el5 = sb.tile([64, TE], BF16, tag="el5", name="el5")
            p_pad = sb.tile([64, 63], BF16, tag="ppad", name="ppad")
            nc.vector.memset(p_pad[:], 0.0)
            for ci, (a, b) in enumerate(((0, 496), (496, TE))):
                bp = dpsum.tile([64, 496], F32, tag="dp", name="bp")
                nc.tensor.matmul(bp[:, 0:b - a], lbct, msa[:, a:b])
                nc.vector.tensor_tensor(h5[:, a:b], h4[:, a:b], bp[:, 0:b - a],
                                        op=AOP.mult)
                nc.scalar.activation(e5[:, a:b], h5[:, a:b], AFT.Exp)
                nc.vector.tensor_scalar(q5[:, a:b], e5[:, a:b], 1.0, 1.0,
                                        op0=AOP.min, op1=AOP.subtract)
                nc.vector.tensor_tensor(el5[:, a:b], h5[:, a:b], q5[:, a:b],
                                        op=AOP.max)
                wa, wb = (7, 38) if ci == 0 else (38, 63)
                with nc.allow_low_precision(reason="bf16 pool in 2e-2 tol"):
                    nc.vector.tensor_reduce(
                        p_pad[:, wa:wb],
                        el5[:, a:b].rearrange("p (w k) -> p w k", k=16),
                        axis=mybir.AxisListType.X, op=AOP.add)

            # ---- separable temporal conv (+BN3 folded): 16 accumulating
            # diagonal matmuls on the (idle) PE
            sep_ps = dpsum.tile([64, 48], F32, tag="dp", name="sep")
            for k in range(PW):
                nc.tensor.matmul(sep_ps[:], dsep[:, 64 * k:64 * (k + 1)],
                                 p_pad[:, k:k + 48],
                                 start=(k == 0), stop=(k == PW - 1))

            # ---- final ELU (fused bias) + pool(16) + /16
            e6 = sb.tile([64, 48], BF16, tag="e6", name="e6")
            nc.scalar.activation(e6[:], sep_ps[:], AFT.Exp, bias=bsept)
            q6 = sb.tile([64, 48], BF16, tag="q6", name="q6")
            nc.vector.tensor_scalar(q6[:], e6[:], 1.0, 1.0,
                                    op0=AOP.min, op1=AOP.subtract)
            el6 = sb.tile([64, 48], F32, tag="el6", name="el6")
            nc.vector.scalar_tensor_tensor(el6[:], sep_ps[:], bsept, q6[:],
                                           op0=AOP.add, op1=AOP.max)
            po = sb.tile([64, 3], F32, tag="po", name="po")
            nc.vector.tensor_reduce(po[:],
                                    el6[:].rearrange("p (w k) -> p w k", k=16),
                                    axis=mybir.AxisListType.X, op=AOP.add)
            ot = sb.tile([64, 3], F32, tag="ot", name="ot")
            nc.vector.tensor_scalar(ot[:], po[:], 1.0 / 16.0, None, op0=AOP.mult)
            nc.sync.dma_start(out_d.ap().rearrange("a b c -> (a b) c"), ot[:])
    nc.compile()
    return nc


def _make_in_maps(f, inputs):
    try:
        import ml_dtypes
        bf = ml_dtypes.bfloat16
    except ImportError:
        bf = np.float32
    pbig = np.zeros((128, 1230), np.float32)
    pbig[:, 0:128] = f['ident']
    pbig[0:64, 128:1152] = f['dsep']
    pbig[0:64, 1152:1230] = f['pb16']
    consts = {
        'kstk': f['kstk'].astype(bf),
        'p32': f['p32'],
        'pbig': pbig.astype(bf),
    }
    wtb = f['wtoep'].astype(bf)
    in_maps = []
    for core in range(NCORE):
        m = dict(consts)
        m['wx'] = np.concatenate(
            [wtb, _host_xtiles(inputs['x'], core).astype(bf)], axis=1)
        in_maps.append(m)
    return in_maps


def kernel(**inputs):
    if 'nc' not in _CACHE:
        _CACHE['nc'] = _build_program()
    nc = _CACHE['nc']
    f = _host_consts(inputs)
    in_maps = _make_in_maps(f, inputs)
    res = run_bass_kernel_spmd(nc, in_maps, list(range(NCORE)))
    out = np.concatenate([np.asarray(res.results[i]['out'])
                          for i in range(NCORE)], axis=0)
    return out.astype(np.float32)


if __name__ == '__main__':
    d = np.load('/root/problem/ref_data.npz')
    inputs = {k: d[k] for k in d.files if k != 'expected'}
    out = kernel(**inputs)
    exp = d['expected']
    err = np.abs(out - exp).max() / (np.abs(exp).max() + 1e-9)
    print('out', out.shape, 'rel(absmax) err', err)
